# revision 1
# baseline (speedup 1.0000x reference)
"""Multi-head attention with 2D RoPE on 8 Trainium2 NeuronCores.

Problem (hardcoded): B=8, L=1024, EMB=768, 12 heads x 64 dim, 2D RoPE
(x/y tables of length 32, base 100), softmax attention, output projection.

Sharding: data-parallel over batch — one batch element per core, no
collectives. Each core computes, all matmuls in float32r (FP22) at full
PE rate:

    qT = (Wq/8)^T @ embT, kT = Wk^T @ embT      (head-transposed layout)
    rope2d via elementwise cos/sin (host-gathered per position) plus a
        16-lane swap done as a PE matmul with a permutation matrix
    v_aug = [v | 1] per head                    (ones column -> softmax sums)
    per head pair, per i-half, per j-tile:
        sT = kT^T @ qT                          ([j, i] layout, 2 heads
                                                 row-packed via tile_position)
        expT = exp(sT)                          (ACT; no max-subtraction:
                                                 |scores| <~ 6 in fp32)
        avT += v_aug^T @ expT                   (row 64 accumulates sums)
    avT_h /= sums_h   (broadcast sums across partitions via a DRAM
                       roundtrip DMA, reciprocal_approx on DVE)
    out = attnout @ Wp + bp   (K=128 accumulation over pairs; bias added
                               during psum drain from a broadcast DMA)

Pipelining: PSUM is split into a shared pool of [128, 1024] slots for
scores/v/final-projection (bufs=2, 4 banks), one slot for the q/k
projection chain (2 banks), and two [65, 512] AV accumulators (2 banks).
Next-pair projections are emitted before the current pair's attention so
PE fills ACT-bound gaps; the steady-state bound is the ACT engine's exp
throughput (~1.2 GHz, 1 elem/lane/cycle over 12.6M attention elements).
"""

import numpy as np

import concourse.bass as bass
import concourse.mybir as mybir
import concourse.tile as tile
from concourse import bacc
from concourse.bass import ts
from concourse.bass_utils import run_bass_kernel_spmd

F32 = mybir.dt.float32
F32R = mybir.dt.float32r
AF = mybir.ActivationFunctionType

HEAD_NUM = 12
EMB = 768
HEAD = 64
L = 1024
B = 8
X_SIZE = 32
Y_SIZE = 32
BASE = 100.0
N_CORES = 8

KT = EMB // 128   # 6 contraction tiles over channels
NJ = L // 128     # 8 position tiles
NPAIR = HEAD_NUM // 2  # 6 head pairs


def _round_f32r(x):
    """Round fp32 to FP22 (e8m13, drop 10 mantissa LSBs, RNE)."""
    v = np.ascontiguousarray(x, dtype=np.float32).view(np.uint32).copy()
    v = v + (np.uint32(0x1FF) + ((v >> np.uint32(10)) & np.uint32(1)))
    v &= np.uint32(0xFFFFFC00)
    return v.view(np.float32)


def _tables_np(pos_len, d, base=BASE):
    inv_freq = 1.0 / base ** (np.arange(0, d, 2, dtype=np.float32) / d)
    freqs = np.outer(np.arange(pos_len, dtype=np.float32), inv_freq)
    freqs = np.concatenate([freqs, freqs], axis=-1)
    return np.sin(freqs).astype(np.float32), np.cos(freqs).astype(np.float32)


def _rope_coeffs(pos):
    """cos128/ssh128: [128, L] elementwise RoPE coefficients, 2 heads deep.

    Row layout per 64-row head block: rows 0:32 x-part, rows 32:64 y-part.
    ssh is the sin table pre-shifted/negated so that
        rope(q) = q * cos128 + R128 @ (q * ssh128)
    where R128 swaps 16-row halves within each 32-row block.
    """
    sx, cx = _tables_np(X_SIZE, HEAD // 2)
    sy, cy = _tables_np(Y_SIZE, HEAD // 2)
    px, py = pos[:, 0], pos[:, 1]
    cosxT = cx[px].T  # [32, L]
    cosyT = cy[py].T
    sinxT = sx[px].T
    sinyT = sy[py].T

    def shift(s):
        out = np.empty_like(s)
        out[0:16] = s[16:32]
        out[16:32] = -s[0:16]
        return out

    cos64 = np.concatenate([cosxT, cosyT], axis=0)          # [64, L]
    ssh64 = np.concatenate([shift(sinxT), shift(sinyT)], axis=0)
    cos128 = np.concatenate([cos64, cos64], axis=0).astype(np.float32)
    ssh128 = np.concatenate([ssh64, ssh64], axis=0).astype(np.float32)
    return np.ascontiguousarray(cos128), np.ascontiguousarray(ssh128)


def _r128():
    r32 = np.zeros((32, 32), dtype=np.float32)
    for d in range(16):
        r32[d, d + 16] = 1.0
        r32[d + 16, d] = 1.0
    return np.kron(np.eye(4, dtype=np.float32), r32)


def build_nc():
    nc = bacc.Bacc()
    embT = nc.declare_dram_parameter("embT", [EMB, L], F32R, isOutput=False)
    wqs = nc.declare_dram_parameter("wqs", [NPAIR, 128, EMB], F32R,
                                    isOutput=False)
    wks = nc.declare_dram_parameter("wks", [NPAIR, 128, EMB], F32R,
                                    isOutput=False)
    wv = nc.declare_dram_parameter("wv", [EMB, EMB], F32R, isOutput=False)
    wp = nc.declare_dram_parameter("wp", [EMB, EMB], F32R, isOutput=False)
    bp = nc.declare_dram_parameter("bp", [1, EMB], F32R, isOutput=False)
    cos = nc.declare_dram_parameter("cos", [128, L], F32, isOutput=False)
    ssh = nc.declare_dram_parameter("ssh", [128, L], F32, isOutput=False)
    r128 = nc.declare_dram_parameter("r128", [128, 128], F32R, isOutput=False)
    onesv = nc.declare_dram_parameter("onesv", [128, HEAD_NUM], F32R,
                                      isOutput=False)
    out = nc.declare_dram_parameter("out", [L, EMB], F32, isOutput=True)

    with tile.TileContext(nc) as tc:
        with (
            tc.tile_pool(name="const", bufs=1) as p_const,
            tc.tile_pool(name="vaug", bufs=1) as p_vaug,
            tc.tile_pool(name="persist", bufs=1) as p_per,
            tc.tile_pool(name="wsl", bufs=2) as p_wsl,
            tc.tile_pool(name="qk", bufs=2) as p_qk,
            tc.tile_pool(name="tmp", bufs=2) as p_tmp,
            tc.tile_pool(name="exp", bufs=6) as p_exp,
            tc.tile_pool(name="norm", bufs=2) as p_norm,
            tc.tile_pool(name="nrm1", bufs=2) as p_nrm1,
            tc.tile_pool(name="outp", bufs=2) as p_out,
            tc.tile_pool(name="big", bufs=2, space="PSUM") as ps_big,
            tc.tile_pool(name="proj", bufs=1, space="PSUM") as ps_proj,
            tc.tile_pool(name="av", bufs=1, space="PSUM") as ps_av,
            tc.tile_pool(name="dram", bufs=4, space="DRAM") as p_dram,
        ):
            # weight slices are pre-swizzled on the host to the exact
            # [128, KT*128] SBUF layout, so each load is one contiguous DMA
            def load_wslice(w_dram, pair, wtag):
                wsl = p_wsl.tile([128, EMB], F32R, tag=wtag,
                                 name=f"wsl{wtag}{pair}")
                nc.sync.dma_start(wsl[:], w_dram[pair])
                return wsl

            # project+rope one head pair's q/k (emitted ahead of its
            # consumers so Tile can overlap across pairs)
            def project_rope(w_dram, pair, wtag, wsl=None, pool=None):
                if wsl is None:
                    wsl = load_wslice(w_dram, pair, wtag)
                if pool is None:
                    qp = ps_proj.tile([128, L], F32, tag="qp",
                                      name=f"qp{wtag}{pair}")
                else:
                    qp = pool.tile([128, L], F32, tag="big",
                                   name=f"qp{wtag}{pair}")
                for c0, c1 in ((0, 512), (512, 1024)):
                    for k in range(KT):
                        nc.tensor.matmul(
                            qp[:, c0:c1],
                            wsl[:, ts(k, 128)],
                            embT_t[k][:, c0:c1],
                            start=(k == 0), stop=(k == KT - 1),
                        )
                t_s = p_tmp.tile([128, L], F32R, tag="ts")
                t_c = p_tmp.tile([128, L], F32, tag="tc")
                dst = p_qk.tile([128, L], F32R, tag=wtag + "T")
                # rope chunk-by-chunk so consumers can start on the first
                # 512 columns early; the rot matmul overwrites qp in place
                for c0, c1 in ((0, 512), (512, 1024)):
                    nc.vector.tensor_mul(t_s[:, c0:c1], qp[:, c0:c1],
                                         ssh_t[:, c0:c1])
                    nc.vector.tensor_mul(t_c[:, c0:c1], qp[:, c0:c1],
                                         cos_t[:, c0:c1])
                    nc.tensor.matmul(qp[:, c0:c1], r_t[:], t_s[:, c0:c1],
                                     start=True, stop=True)
                    nc.vector.tensor_add(dst[:, c0:c1], t_c[:, c0:c1],
                                         qp[:, c0:c1])
                return dst

            # v projection for one j-tile -> v_aug[j]
            def project_v(j):
                vaug3 = vaug_t[j][:].rearrange("p (h d) -> p h d", d=65)
                nc.sync.dma_start(vaug3[:, :, 64:65], onesv[:, :, None])
                vp = ps_big.tile([128, L], F32, tag="big", name=f"vp{j}")
                for c0, c1 in ((0, 512), (512, 768)):
                    for k in range(KT):
                        nc.tensor.matmul(
                            vp[:, c0:c1],
                            embT_t[k][:, ts(j, 128)],
                            wv_t[k][:, c0:c1],
                            start=(k == 0), stop=(k == KT - 1),
                        )
                nc.vector.tensor_copy(
                    vaug3[:, :, 0:64],
                    vp[:, 0:EMB].rearrange("p (h d) -> p h d", d=64))

            # ---- loads: pair-0 weight slices + embT first (the first
            # projection needs them), then rope coefficients, then Wv ----
            PRELOAD = [load_wslice(wqs, 0, "q")]

            # PE warmup: a few throwaway fp32 matmuls during the DMA head
            # keep the HAM clock-gate warm for the first real projections
            wu = p_const.tile([128, 512], F32, tag="warm")
            nc.gpsimd.memset(wu[:], 0.0)
            wup = ps_av.tile([65, 512], F32, tag="av0", name="warmps")
            for _ in range(2):
                nc.tensor.matmul(wup[0:64, :], wu[:, 0:64], wu[:],
                                 start=True, stop=True)

            embT_t = [p_per.tile([128, L], F32R, tag=f"embT{k}",
                                 name=f"embTt{k}") for k in range(KT)]
            for k in range(KT):
                eng = nc.sync if k % 2 == 0 else nc.scalar
                eng.dma_start(embT_t[k][:], embT[ts(k, 128), :])
            cos_t = p_const.tile([128, L], F32, tag="cos")
            ssh_t = p_const.tile([128, L], F32, tag="ssh")
            r_t = p_const.tile([128, 128], F32R, tag="r128")
            wv_t = [p_per.tile([128, EMB], F32R, tag=f"wvp{k}",
                               name=f"wvt{k}") for k in range(KT)]

            nc.sync.dma_start(ssh_t[:], ssh[:])
            nc.sync.dma_start(cos_t[:], cos[:])
            nc.sync.dma_start(r_t[:], r128[:])
            PRELOAD.append(load_wslice(wks, 0, "k"))

            def load_wv():
                for k in range(KT):
                    nc.sync.dma_start(wv_t[k][:], wv[ts(k, 128), :])

            vaug_t = [p_vaug.tile([128, HEAD_NUM * 65], F32R,
                                  tag=f"vaug{j}", name=f"vaug{j}")
                      for j in range(NJ)]
            avT_t = [p_per.tile([128, L], F32R, tag=f"avT{p}", name=f"avT{p}")
                     for p in range(NPAIR)]

            # pair-0 projections first; v(j) is emitted just-in-time inside
            # pair 0's first half-loop so attention starts immediately
            qk_next = (project_rope(wqs, 0, "q", wsl=PRELOAD[0]),
                       project_rope(wks, 0, "k", wsl=PRELOAD[1],
                                    pool=ps_big))
            load_wv()

            # output-projection weights load early (slots shared with wv
            # free up after the v projections)
            wp_t = [p_per.tile([128, EMB], F32R, tag=f"wvp{k}",
                               name=f"wpt{k}") for k in range(KT)]
            for k in range(KT):
                nc.sync.dma_start(wp_t[k][:], wp[ts(k, 128), :])
            bpb_t = p_const.tile([128, EMB], F32R, tag="bpb")
            nc.sync.dma_start(bpb_t[:], bp[:].to_broadcast((128, EMB)))

            for pair in range(NPAIR):
                qT, kT = qk_next
                for half in (0, 512):
                    avp = [ps_av.tile([65, 512], F32, tag=f"av{h}",
                                      name=f"avp{pair}_{half}_{h}")
                           for h in range(2)]
                    for j in range(NJ):
                        if pair == 0 and half == 0:
                            project_v(j)
                        sAB = ps_big.tile([128, L], F32, tag="big",
                                          name=f"s{pair}_{half}_{j}")
                        for h in range(2):
                            p0 = 64 * h
                            nc.tensor.matmul(
                                sAB[:, 512 * h:512 * h + 512],
                                kT[p0:p0 + 64, ts(j, 128)],
                                qT[p0:p0 + 64, half:half + 512],
                                start=True, stop=True,
                                tile_position=(p0, 0),
                            )
                        expt = p_exp.tile([128, L], F32R, tag="expt")
                        nc.scalar.activation(expt[:], sAB[:], AF.Exp)
                        for h in range(2):
                            hh = (2 * pair + h) * 65
                            nc.tensor.matmul(
                                avp[h][:],
                                vaug_t[j][:, hh:hh + 65],
                                expt[:, 512 * h:512 * h + 512],
                                start=(j == 0), stop=(j == NJ - 1),
                            )
                    # drain + normalize this half: rows 0:64 / row 64
                    # (sums). Broadcast the sums row across partitions via
                    # a DRAM roundtrip (partition-stride-0 reads are
                    # DRAM-source only).
                    for h in range(2):
                        av_sb = p_norm.tile([65, 512], F32, tag=f"avsb{h}",
                                            name=f"avsb{pair}_{half}_{h}")
                        nc.vector.tensor_copy(av_sb[:], avp[h][:])
                        sc = p_dram.tile([1, 512], F32, tag="sums")
                        nc.sync.dma_start(sc[:], av_sb[64:65, :])
                        bc = p_nrm1.tile([64, 512], F32, tag="bc")
                        nc.sync.dma_start(bc[:], sc[:].to_broadcast((64, 512)))
                        rb = p_nrm1.tile([64, 512], F32, tag="rb")
                        nc.vector.reciprocal_approx_fast(rb[:], bc[:])
                        nc.gpsimd.tensor_mul(
                            avT_t[pair][64 * h:64 * h + 64,
                                        half:half + 512],
                            av_sb[0:64, :], rb[:])
                    # emit next pair's projections mid-pair so the PE
                    # excursion lands while ACT still has queued exps
                    if half == 0 and pair + 1 < NPAIR:
                        qk_next = (project_rope(wqs, pair + 1, "q"),
                                   project_rope(wks, pair + 1, "k"))

            # ---- output projection: K=128 accumulation over pairs ----
            for i in range(NJ):
                fpool = ps_big if i % 2 == 0 else ps_proj
                fp = fpool.tile([128, L], F32,
                                tag="big" if i % 2 == 0 else "qp",
                                name=f"fp{i}")
                for c0, c1 in ((0, 512), (512, 768)):
                    for pair in range(NPAIR):
                        nc.tensor.matmul(
                            fp[:, c0:c1],
                            avT_t[pair][:, ts(i, 128)],
                            wp_t[pair][:, c0:c1],
                            start=(pair == 0), stop=(pair == NPAIR - 1),
                        )
                o_sb = p_out.tile([128, EMB], F32, tag="osb")
                nc.vector.tensor_add(o_sb[:], fp[:, 0:EMB],
                                     bpb_t[:].bitcast(F32))
                oeng = nc.sync if i % 2 == 0 else nc.scalar
                oeng.dma_start(out[ts(i, 128), :], o_sb[:])

    nc.finalize()
    return nc


_NC_CACHE = {}


def _get_nc(variant=None):
    if variant not in _NC_CACHE:
        _NC_CACHE[variant] = build_nc()
    return _NC_CACHE[variant]


def kernel(emb, pos, Wq, Wk, Wv, Wp, bp, _trace=False, _cores=N_CORES):
    emb = np.asarray(emb, dtype=np.float32)
    pos = np.asarray(pos)
    Wq_s = _round_f32r(np.asarray(Wq, dtype=np.float32) * (HEAD ** -0.5))
    Wk_r = _round_f32r(np.asarray(Wk, dtype=np.float32))
    Wv_r = _round_f32r(np.asarray(Wv, dtype=np.float32))
    Wp_r = _round_f32r(np.asarray(Wp, dtype=np.float32))
    bp2 = _round_f32r(np.asarray(bp, dtype=np.float32).reshape(1, EMB))

    cos128, ssh128 = _rope_coeffs(np.asarray(pos))
    r128 = _r128()
    onesv = np.ones((128, HEAD_NUM), dtype=np.float32)

    def swizzle(w):
        # [EMB, EMB] -> [NPAIR, 128, KT*128]: slice pair columns, gather
        # row t*128+p to partition p, k-tile-major free layout
        return np.ascontiguousarray(
            w.reshape(KT, 128, NPAIR, 128).transpose(2, 1, 0, 3)
            .reshape(NPAIR, 128, EMB))

    nc = _get_nc()
    wqs = swizzle(Wq_s)
    wks = swizzle(Wk_r)
    in_maps = []
    for b in range(_cores):
        in_maps.append({
            "embT": _round_f32r(emb[b].T),
            "wqs": wqs, "wks": wks, "wv": Wv_r, "wp": Wp_r, "bp": bp2,
            "cos": cos128, "ssh": ssh128, "r128": r128,
            "onesv": onesv,
        })
    res = run_bass_kernel_spmd(nc, in_maps, list(range(_cores)), trace=_trace)
    out = np.stack([res.results[b]["out"] for b in range(_cores)], axis=0)
    if _trace:
        return out, res
    return out

